# revision 1
# baseline (speedup 1.0000x reference)
"""Trainium2 Bass kernel for HDGradientCompressionLayer forward.

Reference computation: y = einsum("bsd,df->bsf", x, W) + b
  x: (4, 4096, 1024) f32, W: (1024, 1024) f32, b: (1024,) f32.

Strategy (data-parallel across 8 cores, per sharding hint):
  Flatten x to (16384, 1024); each core gets 2048 rows. Per core the
  kernel computes y_shard = x_shard @ W + b:
    - x rowblock [128, 1024] is cast-loaded f32->bf16 (SWDGE cast DMA),
    - one xbar DMA-transpose produces the 8 stationary [d,row] tiles,
    - 16 bf16 matmuls (N=512, PSUM-accumulated over the 8 d-blocks),
    - DVE adds the (partition-broadcast) f32 bias during PSUM->SBUF
      eviction, stores f32 y rowblock.
  W is cast-loaded once, d-blocked to match the transposed x layout.
"""

import os
from contextlib import ExitStack

import numpy as np

import concourse.bass as bass
import concourse.bacc as bacc
import concourse.tile as tile
from concourse import mybir
from concourse.bass_utils import run_bass_kernel_spmd
from concourse.masks import make_identity

N_CORES = 8
B, S, D = 4, 4096, 1024
F = 1024
ROWS_TOTAL = B * S          # 16384
ROWS = ROWS_TOTAL // N_CORES  # 2048 per core
P = 128
NSPLIT = 512                # one PSUM bank of f32


def build_nc(rows: int = ROWS) -> bass.Bass:
    nc = bacc.Bacc("TRN2", target_bir_lowering=False, debug=False)
    x = nc.dram_tensor("x", [rows, D], mybir.dt.float32, kind="ExternalInput").ap()
    W = nc.dram_tensor("W", [D, F], mybir.dt.float32, kind="ExternalInput").ap()
    b = nc.dram_tensor("b", [F], mybir.dt.float32, kind="ExternalInput").ap()
    y = nc.dram_tensor("y", [rows, F], mybir.dt.float32, kind="ExternalOutput").ap()

    KB = D // P        # 8 contraction blocks
    NB = F // NSPLIT   # 2 psum banks per rowblock
    RB = rows // P     # rowblocks

    with tile.TileContext(nc) as tc, ExitStack() as ctx:
        const = ctx.enter_context(tc.tile_pool(name="const", bufs=1))
        xp = ctx.enter_context(tc.tile_pool(name="xp", bufs=RB))
        xtp = ctx.enter_context(tc.tile_pool(name="xtp", bufs=RB))
        yp = ctx.enter_context(tc.tile_pool(name="yp", bufs=RB))
        psp = ctx.enter_context(tc.tile_pool(name="psp", bufs=1, space="PSUM"))

        # W, cast to bf16, laid out [p, k, f] with d = k*128 + p to match
        # the xbar-transpose output layout of x.  Split per k-block so the
        # first matmuls' weights land early.
        W_bf = const.tile([P, KB, F], mybir.dt.bfloat16)
        W_pkf = W.rearrange("(k p) f -> p k f", p=P)

        # Bias broadcast to all partitions, f32.
        b_bc = const.tile([P, F], mybir.dt.float32)

        # Identity for PE-based transposes.
        ident = const.tile([P, P], mybir.dt.bfloat16)
        make_identity(nc, ident[:])

        # HAM warmup: ~10 cold matmuls (~4.3us of PE activity) on a zeroed
        # tile flip the PE clock gate to 8/8 while the first DMAs land.
        warm = const.tile([P, P], mybir.dt.bfloat16)
        nc.any.memset(warm[:], 0.0)
        warm_ps = psp.tile([P, NSPLIT], mybir.dt.float32, tag="ps0", bufs=3)
        for _ in range(10):
            nc.tensor.matmul(warm_ps[:], warm[:], warm[:, 0:1].to_broadcast([P, NSPLIT]),
                             start=True, stop=True, skip_group_check=True)

        x_tiles = []
        nc.gpsimd.dma_start(W_bf[:, 0, :], W_pkf[:, 0, :])
        for rb in range(2):
            x_bf = xp.tile([P, D], mybir.dt.bfloat16, name="x_bf", tag="x_bf")
            nc.gpsimd.dma_start(x_bf[:], x[rb * P:(rb + 1) * P, :])  # cast load
            x_tiles.append(x_bf)
        nc.gpsimd.dma_start(b_bc[:], b.rearrange("(o f) -> o f", o=1).to_broadcast([P, F]))
        for k in range(1, KB):
            nc.gpsimd.dma_start(W_bf[:, k, :], W_pkf[:, k, :])
        for rb in range(2, RB):
            x_bf = xp.tile([P, D], mybir.dt.bfloat16, name="x_bf", tag="x_bf")
            nc.gpsimd.dma_start(x_bf[:], x[rb * P:(rb + 1) * P, :])
            x_tiles.append(x_bf)

        for rb in range(RB):
            x_bf = x_tiles[rb]
            # Transpose the 8 k-tiles on the PE into one PSUM bank, then one
            # copyback into SBUF.  xT[p, k, j] = x_bf[j, k*128+p].
            psT = psp.tile([P, KB, P], mybir.dt.bfloat16, name="psT", tag="psT", bufs=2)
            for k in range(KB):
                nc.tensor.transpose(psT[:, k, :], x_bf[:, k * P:(k + 1) * P], ident[:])
            xT = xtp.tile([P, KB, P], mybir.dt.bfloat16, name="xT", tag="xT")
            if rb % 2 == 0:
                nc.scalar.copy(xT[:], psT[:])
            else:
                nc.vector.tensor_copy(xT[:], psT[:])

            y_sb = yp.tile([P, F], mybir.dt.float32)
            pss = [psp.tile([P, NSPLIT], mybir.dt.float32, name=f"ps{n}", tag=f"ps{n}", bufs=3) for n in range(NB)]
            for k in range(KB):
                for n in range(NB):
                    nc.tensor.matmul(
                        pss[n][:],
                        xT[:, k, :],
                        W_bf[:, k, n * NSPLIT:(n + 1) * NSPLIT],
                        start=(k == 0),
                        stop=(k == KB - 1),
                    )
            for n in range(NB):
                nc.vector.tensor_add(
                    y_sb[:, n * NSPLIT:(n + 1) * NSPLIT],
                    pss[n][:],
                    b_bc[:, n * NSPLIT:(n + 1) * NSPLIT],
                )
            nc.scalar.dma_start(y[rb * P:(rb + 1) * P, :], y_sb[:])

    nc.compile()
    return nc


_NC_CACHE: dict[int, bass.Bass] = {}


def _get_nc(rows: int = ROWS) -> bass.Bass:
    if rows not in _NC_CACHE:
        _NC_CACHE[rows] = build_nc(rows)
    return _NC_CACHE[rows]


def _run(in_maps, rows: int = ROWS, trace: bool = False):
    nc = _get_nc(rows)
    return run_bass_kernel_spmd(nc, in_maps, list(range(N_CORES)), trace=trace)


def kernel(x: np.ndarray, W: np.ndarray, b: np.ndarray) -> np.ndarray:
    x = np.ascontiguousarray(np.asarray(x, dtype=np.float32))
    W = np.ascontiguousarray(np.asarray(W, dtype=np.float32))
    b = np.ascontiguousarray(np.asarray(b, dtype=np.float32))
    x_flat = x.reshape(ROWS_TOTAL, D)
    in_maps = [
        {"x": np.ascontiguousarray(x_flat[c * ROWS:(c + 1) * ROWS]), "W": W, "b": b}
        for c in range(N_CORES)
    ]
    res = _run(in_maps, trace=bool(int(os.environ.get("BASS_KERNEL_TRACE", "0"))))
    y = np.concatenate([res.results[c]["y"] for c in range(N_CORES)], axis=0)
    return y.reshape(B, S, F)



# revision 2
# speedup vs baseline: 1.1797x; 1.1797x over previous
"""Trainium2 Bass kernel for HDGradientCompressionLayer forward.

Reference computation: y = einsum("bsd,df->bsf", x, W) + b
  x: (4, 4096, 1024) f32, W: (1024, 1024) f32, b: (1024,) f32.

Strategy (data-parallel across 8 cores, per sharding hint):
  Flatten x to (16384, 1024); each core gets 2048 rows (= 16 rowblocks
  of 128).  All layout work happens on the HOST so the device does pure
  HWDGE copy DMAs and the PE does only the 256 bf16 matmuls:
    - host casts x/W to bf16 and pre-transposes each core's x shard to
      xT [d, m]; W and the first m-half of xT are packed into one input
      "wx" [1024, 2048] so a single [128, 2048] DMA per d-block delivers
      both the W k-tile and the x columns the first rowblock groups use,
    - device: for each group of 4 rowblocks (8 PSUM banks), k-outer
      accumulation psum[m,f] += xT[k][:,m-slice].T @ W[k][:,f-slice],
    - DVE adds the (partition-broadcast) f32 bias during PSUM->SBUF
      eviction, scalar(ACT) HWDGE stores the f32 y rowblock.
"""

import os

import numpy as np

import concourse.bass as bass
import concourse.bacc as bacc
import concourse.tile as tile
from concourse import mybir
from concourse.bass_utils import run_bass_kernel_spmd

N_CORES = 8
B, S, D = 4, 4096, 1024
F = 1024
ROWS_TOTAL = B * S            # 16384
ROWS = ROWS_TOTAL // N_CORES  # 2048 per core
P = 128
NSPLIT = 512                  # one PSUM bank of f32
KB = D // P                   # 8 contraction blocks
RB = ROWS // P                # 16 rowblocks per core
GRP = 4                       # rowblocks per PSUM group (4*2 banks = all 8)
MHALF = ROWS // 2             # 1024 columns of xT per input tensor

_BF16 = mybir.dt.np(mybir.dt.bfloat16)


def build_nc() -> bass.Bass:
    nc = bacc.Bacc("TRN2", target_bir_lowering=False, debug=False)
    # wx[d, 0:F] = W[d, :] (bf16);  wx[d, F:F+MHALF] = xT[d, 0:MHALF]
    wx = nc.dram_tensor("wx", [D, F + MHALF], mybir.dt.bfloat16, kind="ExternalInput").ap()
    # xr[d, :] = xT[d, MHALF:ROWS]
    xr = nc.dram_tensor("xr", [D, MHALF], mybir.dt.bfloat16, kind="ExternalInput").ap()
    b = nc.dram_tensor("b", [F], mybir.dt.float32, kind="ExternalInput").ap()
    y = nc.dram_tensor("y", [ROWS, F], mybir.dt.float32, kind="ExternalOutput").ap()

    with tile.TileContext(nc) as tc:
        with tc.tile_pool(name="const", bufs=1) as const, \
             tc.tile_pool(name="ap", bufs=1) as apool, \
             tc.tile_pool(name="bp", bufs=1) as bpool, \
             tc.tile_pool(name="yp", bufs=1) as yp, \
             tc.tile_pool(name="psp", bufs=1, space="PSUM") as psp:

            # Bias broadcast to all partitions, f32 (SWDGE broadcast, tiny).
            b_bc = const.tile([P, F], mybir.dt.float32)
            nc.gpsimd.dma_start(b_bc[:], b.rearrange("(o f) -> o f", o=1).to_broadcast([P, F]))

            # HAM warmup: cold matmuls on a zeroed tile flip the PE clock
            # gate to 8/8 while the first loads land.
            warm = const.tile([P, P], mybir.dt.bfloat16)
            nc.any.memset(warm[:], 0.0)
            warm_ps = psp.tile([P, NSPLIT], mybir.dt.float32, tag="ps_0_0", bufs=1)
            for _ in range(12):
                nc.tensor.matmul(warm_ps[:], warm[:], warm[:, 0:1].to_broadcast([P, NSPLIT]),
                                 start=True, stop=True, skip_group_check=True)

            # Loads on the SP HWDGE ring: 8x 512KB (W k-tile + first x half),
            # then 8x 256KB (second x half).
            wx_t = []
            for k in range(KB):
                t = apool.tile([P, F + MHALF], mybir.dt.bfloat16, name=f"wx{k}", tag=f"wx{k}")
                nc.sync.dma_start(t[:], wx[k * P:(k + 1) * P, :])
                wx_t.append(t)
            xr_t = []
            for k in range(KB):
                t = bpool.tile([P, MHALF], mybir.dt.bfloat16, name=f"xr{k}", tag=f"xr{k}")
                nc.sync.dma_start(t[:], xr[k * P:(k + 1) * P, :])
                xr_t.append(t)

            for g in range(RB // GRP):
                ps = [[psp.tile([P, NSPLIT], mybir.dt.float32, name=f"ps_{j}_{n}",
                                tag=f"ps_{j}_{n}", bufs=1)
                       for n in range(2)] for j in range(GRP)]
                for k in range(KB):
                    for j in range(GRP):
                        m0 = (g * GRP + j) * P
                        if m0 < MHALF:
                            xs = wx_t[k][:, F + m0:F + m0 + P]
                        else:
                            xs = xr_t[k][:, m0 - MHALF:m0 - MHALF + P]
                        for n in range(2):
                            nc.tensor.matmul(
                                ps[j][n][:],
                                xs,
                                wx_t[k][:, n * NSPLIT:(n + 1) * NSPLIT],
                                start=(k == 0),
                                stop=(k == KB - 1),
                            )
                for j in range(GRP):
                    rb = g * GRP + j
                    y_sb = yp.tile([P, F], mybir.dt.float32, name="ysb", tag="ysb", bufs=6)
                    for n in range(2):
                        nc.vector.tensor_add(
                            y_sb[:, n * NSPLIT:(n + 1) * NSPLIT],
                            ps[j][n][:],
                            b_bc[:, n * NSPLIT:(n + 1) * NSPLIT],
                        )
                    nc.scalar.dma_start(y[rb * P:(rb + 1) * P, :], y_sb[:])

    nc.compile()
    return nc


_NC_CACHE: dict[str, bass.Bass] = {}


def _get_nc() -> bass.Bass:
    if "nc" not in _NC_CACHE:
        _NC_CACHE["nc"] = build_nc()
    return _NC_CACHE["nc"]


def _run(in_maps, trace: bool = False):
    nc = _get_nc()
    return run_bass_kernel_spmd(nc, in_maps, list(range(N_CORES)), trace=trace)


def make_in_maps(x: np.ndarray, W: np.ndarray, b: np.ndarray):
    x = np.ascontiguousarray(np.asarray(x, dtype=np.float32)).reshape(ROWS_TOTAL, D)
    W_bf = np.asarray(W, dtype=np.float32).astype(_BF16)
    b = np.ascontiguousarray(np.asarray(b, dtype=np.float32))
    in_maps = []
    for c in range(N_CORES):
        shard_bf = x[c * ROWS:(c + 1) * ROWS].astype(_BF16)  # [2048, 1024]
        wx = np.empty((D, F + MHALF), dtype=_BF16)
        wx[:, :F] = W_bf
        wx[:, F:] = shard_bf[:MHALF].T
        xr = np.ascontiguousarray(shard_bf[MHALF:].T)
        in_maps.append({"wx": wx, "xr": xr, "b": b})
    return in_maps


def kernel(x: np.ndarray, W: np.ndarray, b: np.ndarray) -> np.ndarray:
    in_maps = make_in_maps(x, W, b)
    res = _run(in_maps, trace=bool(int(os.environ.get("BASS_KERNEL_TRACE", "0"))))
    y = np.concatenate([res.results[c]["y"] for c in range(N_CORES)], axis=0)
    return y.reshape(B, S, F)
